# revision 8
# baseline (speedup 1.0000x reference)
"""Trainium2 Bass kernel for nn_CausalityMapBlock.

Math: with p = 1.0 the [B,C,C,F*F] cross tensor collapses algebraically:
  sum_{i,j} (u_i v_j + e)^2 = S2u*S2v + 2e*S1u*S1v + e^2 F^2
  sum_{i,j} (u_i v_j + e)   = S1u*S1v + e F^2
so the whole block reduces to per-channel sums (S1, S2, S1a over F=49
spatial positions) followed by rank-1 outer products over the [C,C] grid.

out[b,m,n] = (num/den + EPS) / LD[n] with
  num = S2[m]S2[n] + 2e S1[m]S1[n] + e^2F^2
  den = S1[m]S1[n] + e F^2
  LD[n] = (S2[n] + 2e S1a[n] + e^2 F)/(S1a[n] + e F) + EPS
Folding 1/LD[n] into the num matmul rhs and dropping the constant terms
that are >1000x below one fp32 ulp of the dominant terms:
  out = num'' * recip(den),  num'' = A2 (x) (A2*iLD) + A1 (x) (3e*A1*iLD)
  den = A1 (x) A1
where A1 = s*sum(x), A2 = s^2*sum(x^2), s = 1/(max_b + EPS).

Sharding: data-parallel over batch B=2; cores 0-3 compute batch 0,
cores 4-7 batch 1 (redundantly within a group; wall-clock identical).
"""

import sys

import numpy as np

for _p in ("/opt/trn_rl_repo",):
    if _p not in sys.path:
        sys.path.insert(0, _p)

EPS = 1e-8
B, C, H, W = 2, 128, 7, 7
F = H * W  # 49
N_CORES = 8

_CACHE = {}


def _build_nc():
    import concourse.bass as bass
    import concourse.bacc as bacc
    import concourse.mybir as mybir
    import concourse.tile as tile

    fp32 = mybir.dt.float32
    # Bacc (not raw Bass): its compile() pass legalizes multi-wait
    # instructions, which this walrus build rejects at codegen otherwise.
    nc = bacc.Bacc("TRN2", target_bir_lowering=False, debug=False)
    xb = nc.dram_tensor("xb", [C, F], fp32, kind="ExternalInput")
    out = nc.dram_tensor("out", [C, C], fp32, kind="ExternalOutput")

    with tile.TileContext(nc) as tc:
        with (
            tc.tile_pool(name="sb", bufs=1) as sb,
            tc.tile_pool(name="ps", bufs=1, space=bass.MemorySpace.PSUM) as ps,
        ):
            ident = sb.tile([128, 128], fp32, tag="ident")
            nc.gpsimd.memset(ident[:], 0.0)
            nc.gpsimd.affine_select(
                out=ident[:], in_=ident[:],
                compare_op=mybir.AluOpType.not_equal,
                fill=1.0, base=0,
                pattern=[[-1, 128]], channel_multiplier=1,
            )

            X = sb.tile([C, F], fp32, tag="X")
            nc.sync.dma_start(X[:], xb[:])

            # per-channel stats columns: [max | sum | sum_sq | sum_abs]
            SC = sb.tile([C, 4], fp32, tag="SC")
            X2 = sb.tile([C, F], fp32, tag="X2")
            nc.vector.reduce_max(SC[:, 0:1], X[:], axis=mybir.AxisListType.X)
            nc.vector.reduce_sum(SC[:, 1:2], X[:], axis=mybir.AxisListType.X)
            nc.scalar.activation(
                X2[:], X[:], mybir.ActivationFunctionType.Square,
                accum_out=SC[:, 2:3],
            )
            nc.vector.reduce_sum(
                SC[:, 3:4], X[:], axis=mybir.AxisListType.X,
                apply_absolute_value=True,
            )

            # transpose each stats column to a [1,128] row on partition 0
            m1t = ps.tile([1, 128], fp32, tag="m1t")
            s1t = ps.tile([1, 128], fp32, tag="s1t")
            s2t = ps.tile([1, 128], fp32, tag="s2t")
            s1at = ps.tile([1, 128], fp32, tag="s1at")
            nc.tensor.transpose(m1t[:], SC[:, 0:1], ident[:])
            nc.tensor.transpose(s1t[:], SC[:, 1:2], ident[:])
            nc.tensor.transpose(s2t[:], SC[:, 2:3], ident[:])
            nc.tensor.transpose(s1at[:], SC[:, 3:4], ident[:])

            # global max -> s = 1/(max+EPS), s2 = s*s
            gmax = sb.tile([1, 1], fp32, tag="gmax")
            s = sb.tile([1, 1], fp32, tag="s")
            s2 = sb.tile([1, 1], fp32, tag="s2")
            nc.vector.reduce_max(gmax[:], m1t[:], axis=mybir.AxisListType.X)
            nc.vector.tensor_scalar_add(gmax[:], gmax[:], float(EPS))
            nc.vector.reciprocal(s[:], gmax[:])
            nc.vector.tensor_mul(s2[:], s[:], s[:])

            # scaled row vectors on partition 0
            A1 = sb.tile([1, 128], fp32, tag="A1")
            A2 = sb.tile([1, 128], fp32, tag="A2")
            A1a = sb.tile([1, 128], fp32, tag="A1a")
            nc.vector.tensor_scalar_mul(A1[:], s1t[:], s[:])
            nc.vector.tensor_scalar_mul(A2[:], s2t[:], s2[:])
            nc.vector.tensor_scalar_mul(A1a[:], s1at[:], s[:])

            # lehmer denominator per channel: LD = (A2 + 2e*A1a)/(A1a + e*F) + e
            nden = sb.tile([1, 128], fp32, tag="nden")
            dden = sb.tile([1, 128], fp32, tag="dden")
            rdd = sb.tile([1, 128], fp32, tag="rdd")
            ld = sb.tile([1, 128], fp32, tag="ld")
            ild = sb.tile([1, 128], fp32, tag="ild")
            nc.vector.scalar_tensor_tensor(
                nden[:], A1a[:], float(2 * EPS), A2[:],
                op0=mybir.AluOpType.mult, op1=mybir.AluOpType.add,
            )
            nc.vector.tensor_scalar_add(dden[:], A1a[:], float(EPS * F))
            nc.vector.reciprocal(rdd[:], dden[:])
            nc.vector.tensor_mul(ld[:], nden[:], rdd[:])
            nc.vector.tensor_scalar_add(ld[:], ld[:], float(EPS))
            nc.vector.reciprocal(ild[:], ld[:])

            # rhs rows with 1/LD folded in
            R0 = sb.tile([1, 128], fp32, tag="R0")
            R1 = sb.tile([1, 128], fp32, tag="R1")
            nc.vector.tensor_mul(R0[:], A2[:], ild[:])
            nc.vector.scalar_tensor_tensor(
                R1[:], A1[:], float(3 * EPS), ild[:],
                op0=mybir.AluOpType.mult, op1=mybir.AluOpType.mult,
            )

            # rank-1 outer products
            nump = ps.tile([128, 128], fp32, tag="nump")
            denp = ps.tile([128, 128], fp32, tag="denp")
            nc.tensor.matmul(nump[:], A2[:], R0[:], start=True, stop=False)
            nc.tensor.matmul(nump[:], A1[:], R1[:], start=False, stop=True)
            nc.tensor.matmul(denp[:], A1[:], A1[:], start=True, stop=True)

            # out = num'' * recip(den)
            rden = sb.tile([128, 128], fp32, tag="rden")
            osb = sb.tile([128, 128], fp32, tag="osb")
            nc.vector.reciprocal(rden[:], denp[:])
            nc.vector.tensor_mul(osb[:], nump[:], rden[:])
            nc.sync.dma_start(out.ap(), osb[:])

    nc.compile()
    return nc


def _get_nc():
    if "nc" not in _CACHE:
        _CACHE["nc"] = _build_nc()
    return _CACHE["nc"]


def kernel(x) -> np.ndarray:
    from concourse.bass_utils import run_bass_kernel_spmd

    x = np.ascontiguousarray(np.asarray(x), dtype=np.float32)
    assert x.shape == (B, C, H, W)
    xf = x.reshape(B, C, F)

    nc = _get_nc()
    in_maps = [{"xb": np.ascontiguousarray(xf[i // 4])} for i in range(N_CORES)]
    res = run_bass_kernel_spmd(nc, in_maps, list(range(N_CORES))).results
    return np.stack([res[0]["out"], res[4]["out"]]).astype(np.float32)


# revision 10
# speedup vs baseline: 1.1583x; 1.1583x over previous
"""Trainium2 Bass kernel for nn_CausalityMapBlock.

Math: with p = 1.0 the [B,C,C,F*F] cross tensor collapses algebraically:
  sum_{i,j} (u_i v_j + e)^2 = S2u*S2v + 2e*S1u*S1v + e^2 F^2
  sum_{i,j} (u_i v_j + e)   = S1u*S1v + e F^2
so the whole block reduces to per-channel sums (S1, S2, S1a over F=49
spatial positions) followed by rank-1 outer products over the [C,C] grid.

With A1 = s*sum(x), A2 = s^2*sum(x^2), A1a = s*sum|x|, s = 1/(max+EPS):
  dd   = A1a + EPS*F            (lehmer denominator's denominator)
  nden = A2 + 2*EPS*A1a         (lehmer denominator's numerator)
  p    = nden + EPS*dd          (folds the +EPS after the ratio)
  out[m,n] = (A2[m]*A2[n]*dd[n] + 3*EPS*A1[m]*A1[n]*dd[n])
           / (A1[m]*A1[n]*p[n])
(constant terms ~1e-13 are >1000x below one fp32 ulp of the dominant
terms and are dropped). Numerator and denominator are rank-1 matmuls of
per-channel vectors; one reciprocal + one multiply finish the job.

All per-channel math runs in column layout [128,1] (full 128-lane DVE
parallelism); a single PE transpose moves the five final vectors to
[5,128] rows for the rank-1 matmuls.

Sharding: data-parallel over batch B=2; cores 0-3 compute batch 0,
cores 4-7 batch 1 (redundantly within a group; wall-clock identical).
"""

import sys

import numpy as np

for _p in ("/opt/trn_rl_repo",):
    if _p not in sys.path:
        sys.path.insert(0, _p)

EPS = 1e-8
B, C, H, W = 2, 128, 7, 7
F = H * W  # 49
N_CORES = 8

_CACHE = {}


def _build_nc():
    import concourse.bass as bass
    import concourse.bacc as bacc
    import concourse.mybir as mybir
    import concourse.tile as tile

    fp32 = mybir.dt.float32
    MUL = mybir.AluOpType.mult
    ADD = mybir.AluOpType.add
    # Bacc (not raw Bass): its compile() pass legalizes multi-wait
    # instructions, which this walrus build rejects at codegen otherwise.
    nc = bacc.Bacc("TRN2", target_bir_lowering=False, debug=False)
    xb = nc.dram_tensor("xb", [C, F], fp32, kind="ExternalInput")
    out = nc.dram_tensor("out", [C, C], fp32, kind="ExternalOutput")

    with tile.TileContext(nc) as tc:
        with (
            tc.tile_pool(name="sb", bufs=1) as sb,
            tc.tile_pool(name="ps", bufs=1, space=bass.MemorySpace.PSUM) as ps,
        ):
            ident = sb.tile([128, 128], fp32, tag="ident")
            nc.gpsimd.memset(ident[:], 0.0)
            nc.gpsimd.affine_select(
                out=ident[:], in_=ident[:],
                compare_op=mybir.AluOpType.not_equal,
                fill=1.0, base=0,
                pattern=[[-1, 128]], channel_multiplier=1,
            )
            ones_row = sb.tile([1, 128], fp32, tag="ones_row")
            nc.vector.memset(ones_row[:], 1.0)

            X = sb.tile([C, F], fp32, tag="X")
            nc.sync.dma_start(X[:], xb[:])

            # per-channel stats, column layout (128-lane parallel)
            mt = sb.tile([C, 1], fp32, tag="mt")
            s1c = sb.tile([C, 1], fp32, tag="s1c")
            s2c = sb.tile([C, 1], fp32, tag="s2c")
            s1ac = sb.tile([C, 1], fp32, tag="s1ac")
            X2 = sb.tile([C, F], fp32, tag="X2")
            nc.vector.reduce_max(mt[:], X[:], axis=mybir.AxisListType.X)
            nc.vector.reduce_sum(s1c[:], X[:], axis=mybir.AxisListType.X)
            nc.vector.scalar_tensor_tensor(
                X2[:], X[:], 1.0, X[:], op0=MUL, op1=MUL, accum_out=s2c[:],
            )
            nc.vector.reduce_sum(
                s1ac[:], X[:], axis=mybir.AxisListType.X,
                apply_absolute_value=True,
            )

            # global max -> s = 1/(max+EPS), broadcast to all partitions
            m1t = ps.tile([1, 128], fp32, tag="m1t")
            nc.tensor.transpose(m1t[:], mt[:], ident[:])
            gmax = sb.tile([1, 1], fp32, tag="gmax")
            sv = sb.tile([1, 1], fp32, tag="sv")
            nc.vector.reduce_max(gmax[:], m1t[:], axis=mybir.AxisListType.X)
            nc.vector.tensor_scalar_add(gmax[:], gmax[:], float(EPS))
            nc.vector.reciprocal(sv[:], gmax[:])
            sbc = ps.tile([C, 1], fp32, tag="sbc")
            nc.tensor.matmul(sbc[:], ones_row[:], sv[:], start=True, stop=True)

            # scaled vectors + lehmer chain, all [128,1] columns.
            # V columns: 0=A1, 1=A2, 2=rhs1, 3=rhs0, 4=rhsD
            V = sb.tile([C, 8], fp32, tag="V")
            a1a = sb.tile([C, 1], fp32, tag="a1a")
            ddc = sb.tile([C, 1], fp32, tag="ddc")
            ndenc = sb.tile([C, 1], fp32, tag="ndenc")
            pc = sb.tile([C, 1], fp32, tag="pc")
            nc.vector.tensor_mul(V[:, 0:1], s1c[:], sbc[:])  # A1
            nc.vector.scalar_tensor_tensor(  # A2 = (S2r*s)*s
                V[:, 1:2], s2c[:], sbc[:], sbc[:], op0=MUL, op1=MUL,
            )
            nc.vector.tensor_mul(a1a[:], s1ac[:], sbc[:])  # A1a
            nc.vector.tensor_scalar_add(ddc[:], a1a[:], float(EPS * F))
            nc.vector.scalar_tensor_tensor(  # nden = A1a*2e + A2
                ndenc[:], a1a[:], float(2 * EPS), V[:, 1:2], op0=MUL, op1=ADD,
            )
            nc.vector.scalar_tensor_tensor(  # p = dd*e + nden
                pc[:], ddc[:], float(EPS), ndenc[:], op0=MUL, op1=ADD,
            )
            nc.vector.scalar_tensor_tensor(  # rhs1 = (A1*3e)*dd
                V[:, 2:3], V[:, 0:1], float(3 * EPS), ddc[:], op0=MUL, op1=MUL,
            )
            nc.vector.tensor_mul(V[:, 3:4], V[:, 1:2], ddc[:])  # rhs0
            nc.vector.tensor_mul(V[:, 4:5], V[:, 0:1], pc[:])  # rhsD

            # transpose to rows; matmul operands must sit at base partition
            # 0 of their own SBUF tiles (lhsT/rhs base must match & be 0)
            lt_ps = ps.tile([2, 128], fp32, tag="lt_ps")
            rt_ps = ps.tile([2, 128], fp32, tag="rt_ps")
            rd_ps = ps.tile([1, 128], fp32, tag="rd_ps")
            nc.tensor.transpose(lt_ps[:], V[:, 0:2], ident[:])  # [A1; A2]
            nc.tensor.transpose(rt_ps[:], V[:, 2:4], ident[:])  # [rhs1; rhs0]
            nc.tensor.transpose(rd_ps[:], V[:, 4:5], ident[:])  # [rhsD]
            LT = sb.tile([2, 128], fp32, tag="LT")
            RT = sb.tile([2, 128], fp32, tag="RT")
            RD = sb.tile([1, 128], fp32, tag="RD")
            nc.vector.tensor_copy(LT[:], lt_ps[:])
            nc.scalar.copy(RT[:], rt_ps[:])
            nc.vector.tensor_copy(RD[:], rd_ps[:])

            # num = A1(x)rhs1 + A2(x)rhs0 as one K=2 matmul; den = A1(x)rhsD
            nump = ps.tile([128, 128], fp32, tag="nump")
            denp = ps.tile([128, 128], fp32, tag="denp")
            nc.tensor.matmul(nump[:], LT[:], RT[:], start=True, stop=True)
            nc.tensor.matmul(
                denp[:], LT[0:1, :], RD[:], start=True, stop=True,
            )

            # out = num * recip(den); inputs are benign positives so the
            # ~51-ulp fast reciprocal is far inside tolerance
            rden = sb.tile([128, 128], fp32, tag="rden")
            osb = sb.tile([128, 128], fp32, tag="osb")
            nc.vector.reciprocal_approx_fast(rden[:], denp[:])
            nc.vector.tensor_mul(osb[:], nump[:], rden[:])
            nc.sync.dma_start(out.ap(), osb[:])

    nc.compile()
    return nc


def _get_nc():
    if "nc" not in _CACHE:
        _CACHE["nc"] = _build_nc()
    return _CACHE["nc"]


def kernel(x) -> np.ndarray:
    from concourse.bass_utils import run_bass_kernel_spmd

    x = np.ascontiguousarray(np.asarray(x), dtype=np.float32)
    assert x.shape == (B, C, H, W)
    xf = x.reshape(B, C, F)

    nc = _get_nc()
    in_maps = [{"xb": np.ascontiguousarray(xf[i // 4])} for i in range(N_CORES)]
    res = run_bass_kernel_spmd(nc, in_maps, list(range(N_CORES))).results
    return np.stack([res[0]["out"], res[4]["out"]]).astype(np.float32)
